# revision 13
# baseline (speedup 1.0000x reference)
"""ChebConv (K=3, lambda_max=2) Bass kernel for 8 Trainium2 NeuronCores.

Math (re_norm = 2/lambda_max = 1, so the X0*(re_norm-1) term vanishes):
    deg   = in-degree(dst) clipped to >= 1;  dinv = deg ** -0.5
    U1[d] = sum_{e: dst_e=d} dinv[src_e] * X0[src_e]        (aggregation 1)
    X1    = -dinv * U1
    U2[d] = sum_{e: dst_e=d} dinv[src_e] * X1[src_e]        (aggregation 2)
    X2    = -2 * dinv * U2 - X0
    out   = relu(X0 @ W0 + X1 @ W1 + X2 @ W2 + b)
          = relu(X0 @ (W0-W2) + X1 @ W1 + (-2*dinv) * (U2 @ W2) + b)

Sharding: nodes are split contiguously across 8 cores (12500 each); each
edge lives on the core owning its dst, sorted by dst and grouped into
128-node dst blocks.  Per block, the segment sum is a weighted one-hot
matmul into PSUM (race-free scatter-add on the tensor engine); the source
rows are fetched with an indirect DMA gather (256 B per edge).  The
cross-core "halo exchange" of X1 between the two aggregations is realised
as two kernel launches with the host concatenating the per-core X1 slices
(host does only integer bookkeeping / slicing / transpose / concat --
every FLOP is on device).
"""

import math
import os

import numpy as np

import concourse.bacc as bacc
import concourse.bass as bass
import concourse.mybir as mybir
from concourse.tile import TileContext

P = 128
NCORES = 8
F = 64          # feature width (in_feats = out_feats = 64)
SB_BLOCKS = 7   # dst blocks per superblock (one gather per superblock)

fp32 = mybir.dt.float32
i32 = mybir.dt.int32

TRACE = os.environ.get("CHEB_TRACE", "0") == "1"
LAST_RESULTS = []  # run_bass_kernel_spmd results of the last kernel() call


# --------------------------------------------------------------------------
# host-side integer bookkeeping (graph structure only -- no float math)
# --------------------------------------------------------------------------
RANGE = 32768  # dma_gather idx is int16: each gather call covers one range


def _prep(src, dst, N):
    """Partition edges by dst core; within a core group them by
    (superblock, src-range, dst-block), each (block, range) segment padded
    to whole 128-edge tiles with tile counts uniform across cores (one
    SPMD program).  dma_gather wants int16 range-local indices wrapped in
    16 partitions and replicated 8x down the 128 partitions."""
    npc = N // NCORES
    assert npc * NCORES == N
    nblk = math.ceil(npc / P)
    R = math.ceil(N / RANGE)
    n_super = math.ceil(nblk / SB_BLOCKS)

    deg = np.bincount(dst, minlength=N).astype(np.int32)

    core = dst // npc
    loc = dst % npc
    blk = loc // P
    off = loc % P
    rng_e = src // RANGE

    # edges per (core, block, range)
    cnt = np.bincount(
        (core * nblk + blk) * R + rng_e, minlength=NCORES * nblk * R
    ).reshape(NCORES, nblk, R)
    tiles_br = -(-cnt.max(axis=0) // P)  # [nblk, R], uniform across cores

    # global tile ordering: superblock-major, then range, then block
    tile_start = np.zeros((nblk, R), np.int64)
    spans = []  # spans[s][r] = (tile0, ntiles) of gather call (s, r)
    t = 0
    for s in range(n_super):
        b0, b1 = s * SB_BLOCKS, min((s + 1) * SB_BLOCKS, nblk)
        spans_s = []
        for r in range(R):
            tr0 = t
            for b in range(b0, b1):
                tile_start[b, r] = t
                t += int(tiles_br[b, r])
            spans_s.append((tr0, t - tr0))
        spans.append(spans_s)
    T = t
    tiles_of_block = [
        [
            int(tile_start[b, r]) + k
            for r in range(R)
            for k in range(int(tiles_br[b, r]))
        ]
        for b in range(nblk)
    ]

    g_idx = np.zeros((NCORES, T * P), np.int16)  # range-local src row
    d_off = np.full((NCORES, T * P), -1.0, np.float32)
    d_src = np.ones((NCORES, T * P), np.int32)

    order = np.argsort(dst, kind="stable")
    core_sorted = core[order]
    tsf = (tile_start.reshape(-1) * P).astype(np.int64)  # slot base per (b, r)
    for c in range(NCORES):
        oc = order[core_sorted == c]
        key = blk[oc] * R + rng_e[oc]
        o2 = np.argsort(key, kind="stable")
        oc = oc[o2]
        key = key[o2]
        counts = np.bincount(key, minlength=nblk * R)
        starts = np.concatenate([[0], np.cumsum(counts)[:-1]])
        rank = np.arange(len(oc)) - starts[key]
        pos = tsf[key] + rank
        g_idx[c, pos] = (src[oc] - rng_e[oc] * RANGE).astype(np.int16)
        d_off[c, pos] = off[oc]
        d_src[c, pos] = deg[src[oc]]

    # [T*P] slot order is (tile, partition); doff/dsrc go to [P, T]
    d_off = np.ascontiguousarray(d_off.reshape(NCORES, T, P).transpose(0, 2, 1))
    d_src = np.ascontiguousarray(d_src.reshape(NCORES, T, P).transpose(0, 2, 1))
    # idx tensor: [128, 8*T] int16 -- slot j of a gather call lands at
    # [j % 16, call_col0 + j // 16], replicated down the partition groups.
    g16 = g_idx.reshape(NCORES, T * P // 16, 16).transpose(0, 2, 1)  # [8,16,8T]
    g16 = np.ascontiguousarray(np.tile(g16, (1, 8, 1)))  # [NCORES, 128, 8T]

    degd = np.ones((NCORES, nblk * P), np.int32)
    degd[:, :npc] = deg.reshape(NCORES, npc)
    degd = np.ascontiguousarray(degd.reshape(NCORES, nblk, P).transpose(0, 2, 1))

    return dict(
        npc=npc,
        nblk=nblk,
        R=R,
        spans=spans,
        tiles_of_block=tiles_of_block,
        T=T,
        g_idx=g16,
        d_off=d_off,
        d_src=d_src,
        degd=degd,
    )


def _iota_tile():
    return np.tile(np.arange(P, dtype=np.float32), (P, 1))


# --------------------------------------------------------------------------
# device kernel builder (one SPMD program, identical on every core)
# --------------------------------------------------------------------------
def _build(N, meta, second):
    npc = meta["npc"]
    nblk = meta["nblk"]
    R = meta["R"]
    spans = meta["spans"]
    tiles_of_block = meta["tiles_of_block"]
    T = meta["T"]
    nc = bacc.Bacc()

    xsrc = nc.dram_tensor("xsrc", [N, F], fp32, kind="ExternalInput")
    gidx = nc.dram_tensor("gidx", [P, 8 * T], mybir.dt.int16, kind="ExternalInput")
    doff = nc.dram_tensor("doff", [P, T], fp32, kind="ExternalInput")
    dsrc = nc.dram_tensor("dsrc", [P, T], i32, kind="ExternalInput")
    degd = nc.dram_tensor("degd", [P, nblk], i32, kind="ExternalInput")
    iota = nc.dram_tensor("iota", [P, P], fp32, kind="ExternalInput")
    if second:
        x0T = nc.dram_tensor("x0T", [F, npc], fp32, kind="ExternalInput")
        x1T = nc.dram_tensor("x1T", [F, npc], fp32, kind="ExternalInput")
        w_in = nc.dram_tensor("w_in", [3 * F, F], fp32, kind="ExternalInput")
        bias = nc.dram_tensor("bias", [P, F], fp32, kind="ExternalInput")
        t_out = nc.dram_tensor("out", [npc, F], fp32, kind="ExternalOutput")
    else:
        t_out = nc.dram_tensor("x1out", [npc, F], fp32, kind="ExternalOutput")

    n_super = math.ceil(nblk / SB_BLOCKS)

    with TileContext(nc) as tc:
        with (
            tc.tile_pool(name="const", bufs=1) as cp,
            tc.tile_pool(name="meta", bufs=2) as mp,
            tc.tile_pool(name="gath", bufs=2) as gp,
            tc.tile_pool(name="onehot", bufs=6) as lp,
            tc.tile_pool(name="outs", bufs=3) as op,
            tc.tile_pool(name="psum", bufs=2, space="PSUM") as pp,
        ):
            iota_sb = cp.tile([P, P], fp32)
            nc.sync.dma_start(out=iota_sb[:], in_=iota[:])

            # dinv of this core's dst nodes, laid out [P, nblk]
            degd_sb = cp.tile([P, nblk], i32)
            nc.sync.dma_start(out=degd_sb[:], in_=degd[:])
            degf = cp.tile([P, nblk], fp32)
            nc.vector.tensor_scalar(
                out=degf[:], in0=degd_sb[:], scalar1=1, scalar2=None,
                op0=mybir.AluOpType.max,
            )
            sqd = cp.tile([P, nblk], fp32)
            nc.scalar.activation(sqd[:], degf[:], mybir.ActivationFunctionType.Sqrt)
            dinv_d = cp.tile([P, nblk], fp32)
            nc.vector.reciprocal(dinv_d[:], sqd[:])

            if second:
                w_sb = cp.tile([F, 3, F], fp32)  # [f, k, o] = W[k*F+f, o]
                nc.sync.dma_start(
                    out=w_sb[:],
                    in_=w_in[:].rearrange("(k f) o -> f k o", f=F),
                )
                wp = cp.tile([F, F], fp32)  # W0 - W2
                nc.vector.tensor_tensor(
                    out=wp[:], in0=w_sb[:, 0, :], in1=w_sb[:, 2, :],
                    op=mybir.AluOpType.subtract,
                )
                bias_sb = cp.tile([P, F], fp32)
                nc.sync.dma_start(out=bias_sb[:], in_=bias[:])

            for s in range(n_super):
                b0 = s * SB_BLOCKS
                b1 = min(b0 + SB_BLOCKS, nblk)
                nb = b1 - b0
                t0 = int(spans[s][0][0])
                Ts = int(spans[s][-1][0] + spans[s][-1][1] - t0)
                c0 = b0 * P
                cw = min(npc, b1 * P) - c0  # valid node-columns here

                gidx_sb = mp.tile([P, 8 * Ts], mybir.dt.int16, tag="gidx")
                nc.sync.dma_start(
                    out=gidx_sb[:], in_=gidx[:, 8 * t0 : 8 * (t0 + Ts)]
                )
                doff_sb = mp.tile([P, Ts], fp32, tag="doff")
                nc.sync.dma_start(out=doff_sb[:], in_=doff[:, t0 : t0 + Ts])
                dsrc_sb = mp.tile([P, Ts], i32, tag="dsrc")
                nc.sync.dma_start(out=dsrc_sb[:], in_=dsrc[:, t0 : t0 + Ts])

                # per-edge weight 1/sqrt(max(deg_src, 1))
                wf = mp.tile([P, Ts], fp32, tag="wf")
                nc.vector.tensor_scalar(
                    out=wf[:], in0=dsrc_sb[:], scalar1=1, scalar2=None,
                    op0=mybir.AluOpType.max,
                )
                wsq = mp.tile([P, Ts], fp32, tag="wsq")
                nc.scalar.activation(
                    wsq[:], wf[:], mybir.ActivationFunctionType.Sqrt
                )
                w_e = mp.tile([P, Ts], fp32, tag="we")
                nc.vector.reciprocal(w_e[:], wsq[:])

                # gather all source rows of this superblock (256 B each),
                # one dma_gather per int16 source range
                y_sb = gp.tile([P, Ts, F], fp32, tag="y")
                for r in range(R):
                    tr0, ntr = spans[s][r]
                    if ntr == 0:
                        continue
                    tl = int(tr0 - t0)
                    rb = r * RANGE
                    rl = min(N, rb + RANGE) - rb
                    nc.gpsimd.dma_gather(
                        out_ap=y_sb[:, tl : tl + ntr, :],
                        in_ap=xsrc[rb : rb + rl, :],
                        idxs_ap=gidx_sb[:, 8 * tl : 8 * (tl + ntr)],
                        num_idxs=ntr * P,
                        num_idxs_reg=ntr * P,
                        elem_size=F,
                        single_packet=False,
                    )

                if second:
                    x0T_sb = mp.tile([F, SB_BLOCKS * P], fp32, tag="x0T")
                    nc.sync.dma_start(out=x0T_sb[:, :cw], in_=x0T[:, c0 : c0 + cw])
                    x1T_sb = mp.tile([F, SB_BLOCKS * P], fp32, tag="x1T")
                    nc.sync.dma_start(out=x1T_sb[:, :cw], in_=x1T[:, c0 : c0 + cw])

                o_sb = op.tile([P, nb, F], fp32, tag="osb")

                for bl in range(nb):
                    b = b0 + bl
                    tlist = [g - t0 for g in tiles_of_block[b]]
                    nt = len(tlist)
                    vb = min(P, npc - b * P)  # valid rows in this block

                    if not second:
                        # U1 in [dst, feat] orientation
                        ps = pp.tile([P, F], fp32, tag="agg", space="PSUM")
                        for k, t in enumerate(tlist):
                            L = lp.tile([P, P], fp32, tag="L")
                            nc.vector.tensor_scalar(
                                out=L[:], in0=iota_sb[:],
                                scalar1=doff_sb[:, t : t + 1],
                                scalar2=w_e[:, t : t + 1],
                                op0=mybir.AluOpType.is_equal,
                                op1=mybir.AluOpType.mult,
                            )
                            nc.tensor.matmul(
                                ps[:], lhsT=L[:], rhs=y_sb[:, t, :],
                                start=(k == 0), stop=(k == nt - 1),
                            )
                        # X1 = -dinv * U1
                        nc.vector.tensor_scalar(
                            out=o_sb[:vb, bl, :], in0=ps[:vb, :],
                            scalar1=dinv_d[:vb, b : b + 1], scalar2=-1.0,
                            op0=mybir.AluOpType.mult, op1=mybir.AluOpType.mult,
                        )
                    else:
                        # U2 in transposed [feat, dst] orientation
                        psT = pp.tile([F, P], fp32, tag="aggT", space="PSUM")
                        for k, t in enumerate(tlist):
                            L = lp.tile([P, P], fp32, tag="L")
                            nc.vector.tensor_scalar(
                                out=L[:], in0=iota_sb[:],
                                scalar1=doff_sb[:, t : t + 1],
                                scalar2=w_e[:, t : t + 1],
                                op0=mybir.AluOpType.is_equal,
                                op1=mybir.AluOpType.mult,
                            )
                            nc.tensor.matmul(
                                psT[:], lhsT=y_sb[:, t, :], rhs=L[:],
                                start=(k == 0), stop=(k == nt - 1),
                            )
                        u2T = op.tile([F, P], fp32, tag="u2T")
                        nc.vector.tensor_copy(out=u2T[:], in_=psT[:])

                        # unscaled part: X0 @ (W0-W2) + X1 @ W1
                        pso = pp.tile([P, F], fp32, tag="dense", space="PSUM")
                        nc.tensor.matmul(
                            pso[:vb, :], lhsT=x0T_sb[:, bl * P : bl * P + vb],
                            rhs=wp[:], start=True, stop=False,
                        )
                        nc.tensor.matmul(
                            pso[:vb, :], lhsT=x1T_sb[:, bl * P : bl * P + vb],
                            rhs=w_sb[:, 1, :], start=False, stop=True,
                        )
                        # scaled part: U2 @ W2, then * (-2 * dinv)
                        pst = pp.tile([P, F], fp32, tag="t2", space="PSUM")
                        nc.tensor.matmul(
                            pst[:vb, :], lhsT=u2T[:, :vb],
                            rhs=w_sb[:, 2, :], start=True, stop=True,
                        )
                        t2 = op.tile([P, F], fp32, tag="t2sb")
                        nc.vector.tensor_scalar(
                            out=t2[:vb, :], in0=pst[:vb, :],
                            scalar1=dinv_d[:vb, b : b + 1], scalar2=-2.0,
                            op0=mybir.AluOpType.mult, op1=mybir.AluOpType.mult,
                        )
                        acc = op.tile([P, F], fp32, tag="accsb")
                        nc.vector.tensor_tensor(
                            out=acc[:vb, :], in0=pso[:vb, :], in1=t2[:vb, :],
                            op=mybir.AluOpType.add,
                        )
                        nc.vector.tensor_tensor(
                            out=acc[:vb, :], in0=acc[:vb, :], in1=bias_sb[:vb, :],
                            op=mybir.AluOpType.add,
                        )
                        nc.vector.tensor_scalar(
                            out=o_sb[:vb, bl, :], in0=acc[:vb, :],
                            scalar1=0.0, scalar2=None, op0=mybir.AluOpType.max,
                        )

                # one DMA out per superblock (plus a remainder row write)
                full = cw // P
                if full:
                    nc.sync.dma_start(
                        out=t_out[c0 : c0 + full * P, :].rearrange(
                            "(b p) f -> p b f", p=P
                        ),
                        in_=o_sb[:, :full, :],
                    )
                rem = cw - full * P
                if rem:
                    nc.sync.dma_start(
                        out=t_out[c0 + full * P : c0 + cw, :],
                        in_=o_sb[:rem, full, :],
                    )

    nc.compile()
    return nc


# --------------------------------------------------------------------------
# host pipeline
# --------------------------------------------------------------------------
def _in_maps_first(meta, feat):
    iota = _iota_tile()
    return [
        {
            "xsrc": np.ascontiguousarray(feat),
            "gidx": meta["g_idx"][c],
            "doff": meta["d_off"][c],
            "dsrc": meta["d_src"][c],
            "degd": meta["degd"][c],
            "iota": iota,
        }
        for c in range(NCORES)
    ]


def _in_maps_second(meta, feat, x1_full, W, b):
    npc = meta["npc"]
    iota = _iota_tile()
    featT = np.ascontiguousarray(feat.T)          # [F, N]
    x1T = np.ascontiguousarray(x1_full.T)         # [F, N]
    bias_tile = np.tile(b.astype(np.float32), (P, 1))
    return [
        {
            "xsrc": x1_full,
            "gidx": meta["g_idx"][c],
            "doff": meta["d_off"][c],
            "dsrc": meta["d_src"][c],
            "degd": meta["degd"][c],
            "iota": iota,
            "x0T": np.ascontiguousarray(featT[:, c * npc : (c + 1) * npc]),
            "x1T": np.ascontiguousarray(x1T[:, c * npc : (c + 1) * npc]),
            "w_in": np.ascontiguousarray(W.astype(np.float32)),
            "bias": bias_tile,
        }
        for c in range(NCORES)
    ]


def _install_axon_ntff_hook():
    """This image's antenv lacks axon_hooks; synthesize it from the boot
    helper so trace=True can capture NTFF profiles. No-op if present."""
    import sys
    import types

    try:
        from antenv.axon_hooks import get_axon_ntff_profile_hook  # noqa: F401

        return
    except ImportError:
        pass
    try:
        try:
            from trn_agent_boot.trn_boot import _ntff_profile_via_ctypes
        except ImportError:
            sys.path.insert(0, "/root/.axon_site")
            from trn_agent_boot.trn_boot import _ntff_profile_via_ctypes

        hook = _ntff_profile_via_ctypes("/opt/axon/libaxon_pjrt.so")
    except Exception:
        hook = None
    import antenv

    mod = types.ModuleType("antenv.axon_hooks")
    mod.get_axon_ntff_profile_hook = lambda: hook
    sys.modules["antenv.axon_hooks"] = mod
    antenv.axon_hooks = mod


def _run_hw(nc, in_maps):
    if TRACE:
        _install_axon_ntff_hook()
    from concourse.bass_utils import run_bass_kernel_spmd

    res = run_bass_kernel_spmd(
        nc, in_maps, core_ids=list(range(NCORES)), trace=TRACE
    )
    LAST_RESULTS.append(res)
    return res.results


def _pipeline(feat, src, dst, W, b, runner):
    N = feat.shape[0]
    feat = np.asarray(feat, dtype=np.float32)
    src = np.asarray(src).astype(np.int64)
    dst = np.asarray(dst).astype(np.int64)
    W = np.asarray(W, dtype=np.float32)
    b = np.asarray(b, dtype=np.float32)

    meta = _prep(src, dst, N)
    npc = meta["npc"]

    nc1 = _build(N, meta, second=False)
    outs1 = runner(nc1, _in_maps_first(meta, feat))
    x1_full = np.ascontiguousarray(
        np.concatenate([outs1[c]["x1out"] for c in range(NCORES)], axis=0)
    )

    nc2 = _build(N, meta, second=True)
    outs2 = runner(nc2, _in_maps_second(meta, feat, x1_full, W, b))
    out = np.concatenate([outs2[c]["out"] for c in range(NCORES)], axis=0)
    return np.ascontiguousarray(out)


def kernel(**inputs):
    LAST_RESULTS.clear()
    return _pipeline(
        inputs["feat"], inputs["src"], inputs["dst"], inputs["W"], inputs["b"],
        _run_hw,
    )
